# revision 1
# baseline (speedup 1.0000x reference)
"""Trainium2 raw-Bass kernel v2 for nn_DualAttentionModule (dual attention).

Reference (B=2, C=128, H=W=64, HW=4096):
  pos  = h1x1(x) @ softmax(f1x1(x)^T g1x1(x), rows)^T + x
  chan = x @ softmax(x^T x, rows) + x          (per batch, x as (C, HW))
  y    = W1 @ pos + W2 @ chan + out_b          (out_w = [W1 | W2])

Sharding: 8 cores = 2 batches x 4 query-quarters (NQ=1024 queries each).

v2 design: ACT (exp) is the tallest engine pole (~67us/core); keep it
saturated and keep PE dense behind it.
  - f/g convs folded on host into mfg = f_w^T g_w; Q' = mfg^T xq on device.
    Per-column bias v_j = f_b . g(xc)_j folded into the pos exp bias (exact).
  - chan exp 2048-wide supertiles, half-major (xc second half needed only at
    s=8); accum_out rowsums per-(it,half).
  - pos exp 1024-wide singles; all 32 Lt tiles paced through the back half
    (chan AV split into half-groups so an Lt lands every ~850ns).
  - rowsums: DVE (evens + t31) and Pool (odds + t30) f32 accumulators,
    reduced by an accumulating ones-matmul on PE, scheduled mid-pav-burst.
  - bf16 for chanP/ptb/vpt/xnt/chanacc and all AV matmuls.
  - Pool: PSUM->SBUF drains (vpt/qsb/slab0/chanacc/obs) + odd racc + h1 tail.
PSUM: PL1/PL2, two [128, 4, 512] tensors (4 banks each), hand-rotated:
  chan supertiles 2-deep (even s -> PL2, odd -> PL1); back half: Lt slots
  A=PL2[0:2]/B=PL2[2:4] 2-deep, cav+pav rotate PL1[0:2], w2+rred PL1[2:4].
"""

import numpy as np
from ml_dtypes import bfloat16

C = 128
HW = 4096
NQ = 1024
NIT = 8              # i-tiles per core (chan)
NJT = 32             # j-tiles (pos)
NST = 16             # chan exp supertiles (2048 wide, half-major)
POS_OFF = 90.0

_CACHE = {}


def _build_bass(repeat=1):
    from contextlib import ExitStack

    import concourse.bass as bass
    import concourse.mybir as mybir

    f32 = mybir.dt.float32
    f32r = mybir.dt.float32r
    bf16 = mybir.dt.bfloat16
    Exp = mybir.ActivationFunctionType.Exp
    add = mybir.AluOpType.add
    mult = mybir.AluOpType.mult
    X = mybir.AxisListType.X

    nc = bass.Bass(dynamic_dma_scratch_size=8192)

    # ---- DRAM params ----
    xq_d = nc.declare_dram_parameter("xq", [C, NQ], f32, isOutput=False)
    xc_d = nc.declare_dram_parameter("xc", [C, HW], f32, isOutput=False)
    vptb_d = nc.declare_dram_parameter("vptb", [HW, C], bf16, isOutput=False)
    xtb_d = nc.declare_dram_parameter("xtb", [NQ, C], bf16, isOutput=False)
    mfg_d = nc.declare_dram_parameter("mfg", [C, C], f32, isOutput=False)
    w12T_d = nc.declare_dram_parameter("w12T", [C, C], f32, isOutput=False)
    w2T_d = nc.declare_dram_parameter("w2T", [C, C], bf16, isOutput=False)
    negv_d = nc.declare_dram_parameter("negv", [128, NJT], f32, isOutput=False)
    minh_d = nc.declare_dram_parameter("minh", [128, NIT], f32, isOutput=False)
    bslab_d = nc.declare_dram_parameter("bslab", [C, 1], f32, isOutput=False)
    onesf_d = nc.declare_dram_parameter("ones_f", [128, 1], f32, isOutput=False)
    onesr_d = nc.declare_dram_parameter("ones_r", [1, 128], f32, isOutput=False)
    out_slab_d = nc.declare_dram_parameter("out_slab", [C, NQ], f32, isOutput=True)
    out_chan_d = nc.declare_dram_parameter("out_chan", [C, HW], f32, isOutput=True)

    # ---- SBUF map (bytes 0..8192 pinned DMA scratch) ----
    off = [8192]

    def at(name, shape, dtype):
        h = nc.alloc_sbuf_tensor_at(name, shape, dtype, offset=off[0])
        esz = 2 if dtype == bf16 else 4
        sz = int(np.prod(shape[1:])) * esz
        off[0] += (sz + 31) // 32 * 32
        return h[:]

    xq = at("xq_sb", [C, NQ], f32r)                  # 4K
    xc = at("xc_sb", [C, HW], f32r)                  # 16K
    xtb = at("xtb_sb", [128, NIT, C], bf16)          # 2K
    qsb = at("qsb", [C, NQ], f32r)                   # 4K
    slab0 = at("slab0", [C, NQ], f32)                # 4K
    chanP_off = off[0]
    chanP = at("chanP", [128, NIT, HW], bf16)        # 64K
    ptb = at("ptb", [128, NJT, NQ], bf16)            # 64K
    vpt = at("vpt", [128, NJT, C], bf16)             # 8K
    racc_p = at("racc_p", [128, NQ], f32r)           # 4K
    rs2 = at("rs2", [128, NIT, 2], f32)
    rc = at("rc", [128, NIT], f32)
    rcr = at("rcr", [128, NIT], f32)
    xnt = at("xnt", [128, NIT, C], bf16)             # 2K
    chanacc = at("chanacc", [C, HW], bf16)           # 8K
    obs = at("obs", [C, 8, 512], f32)                # 16K (no rotation)
    slab = at("slab", [C, NQ], f32)                  # 4K
    mfg = at("mfg_sb", [C, C], f32r)
    w12T = at("w12T_sb", [C, C], f32r)
    w2T = at("w2T_sb", [C, C], bf16)
    negv = at("negv_sb", [128, NJT], f32)
    minh = at("minh_sb", [128, NIT], f32)
    bslab = at("bslab_sb", [C, 1], f32)
    ones_f = at("ones_f_sb", [128, 1], f32r)
    onesr = at("onesr_sb", [1, 128], f32r)
    wbuf = at("wbuf", [128, 640], f32)
    wbufa = at("wbufa", [128, 4], f32)
    # rrec_f aliases the first 4KB of chanP (partition-0 row; chanP's next
    # writer is cexp0 of the following repeat, ordered via the ACT barrier)
    rrec_f = nc.alloc_sbuf_tensor_at("rrec_f", [1, NQ], f32r, offset=chanP_off)[:]
    # rrsb (replicated recip, SBUF) also aliases chanP (tail-only lifetime)
    rrsb = nc.alloc_sbuf_tensor_at("rrsb", [128, NQ], f32, offset=chanP_off + 4096)[:]
    assert off[0] <= nc.SBUF_PARTITION_SIZE_BYTES, off[0]

    def flat(ap):
        return ap.rearrange("p a b -> p (a b)")

    # ---- static schedule tables ----
    pe_seq = [("cqk", s) for s in range(NST)]
    pe_seq += [("qprime", 0), ("w12", 0)]
    # back half: lt tiles paced between cav half-groups / w2 / pav
    pe_seq += [("lt", 0), ("lt", 1)]
    nlt = 2
    for jc in range(8):
        pe_seq.append(("cava", jc))
        pe_seq.append(("lt", nlt)); nlt += 1
        pe_seq.append(("cavb", jc))
        pe_seq.append(("lt", nlt)); nlt += 1
        if jc >= 2:
            pe_seq.append(("w2", jc - 2))
    pe_seq += [("w2", 6), ("lt", nlt)]; nlt += 1
    pe_seq += [("w2", 7)]
    npav = 0
    first = True
    while nlt < NJT:               # lt19..31 with pavs interleaved (denser)
        pe_seq.append(("lt", nlt)); nlt += 1
        pe_seq.append(("pav", npav)); npav += 1
        if not first and npav <= nlt - 4:
            pe_seq.append(("pav", npav)); npav += 1
        first = False
    while npav < NJT:
        pe_seq.append(("pav", npav)); npav += 1
    pe_seq += [("rred", 0), ("rrep", 0)]
    p_val = {k: i + 1 for i, k in enumerate(pe_seq)}

    act_seq = [("cexp", s) for s in range(NST)]
    act_seq += [("pexp", t) for t in range(NJT)]
    act_seq += [("rrsb1", 0)]
    a_val = {k: i + 1 for i, k in enumerate(act_seq)}

    dve_seq = [("qsb", 0), ("rs", 0), ("rcr", 0)]
    dve_seq += [("xnt", t) for t in range(NIT)]
    dve_seq += [("slab0", 0)]
    dve_seq += [("cacc", 0), ("cacc", 1), ("obs", 0), ("cacc", 2), ("obs", 1),
                ("cacc", 3), ("obs", 2), ("cacc", 4), ("obs", 3), ("cacc", 5),
                ("obs", 4), ("cacc", 6), ("obs", 5), ("cacc", 7), ("obs", 6),
                ("obs", 7)]
    dve_seq += [("recip", 0), ("rrsb", 0), ("smul", 0), ("smul1", 0), ("sadd", 0)]
    v_val = {k: i + 1 for i, k in enumerate(dve_seq)}

    pool_seq = [("racc", t) for t in range(NJT)]
    pool_seq += [("sadd1", 0)]
    l_val = {k: i + 1 for i, k in enumerate(pool_seq)}

    P_TOT = len(pe_seq)
    A_TOT = len(act_seq)
    V_TOT = len(dve_seq)
    L_TOT = len(pool_seq)
    O_PER = 8 * 16   # 8 chan chunks on SP; 2 slab halves count on SO2

    with ExitStack() as ctx:
        PL1 = ctx.enter_context(nc.psum_tensor("PL1", [128, 4, 512], f32))[:]
        PL2 = ctx.enter_context(nc.psum_tensor("PL2", [128, 4, 512], f32))[:]
        SDq = ctx.enter_context(nc.semaphore("SDq"))
        SDm = ctx.enter_context(nc.semaphore("SDm"))
        SDa = ctx.enter_context(nc.semaphore("SDa"))
        SDb = ctx.enter_context(nc.semaphore("SDb"))
        SDb2 = ctx.enter_context(nc.semaphore("SDb2"))
        SDc = ctx.enter_context(nc.semaphore("SDc"))
        SDd = ctx.enter_context(nc.semaphore("SDd"))
        SDe = ctx.enter_context(nc.semaphore("SDe"))
        SP_ = ctx.enter_context(nc.semaphore("SPE"))
        SA = ctx.enter_context(nc.semaphore("SA"))
        SV = ctx.enter_context(nc.semaphore("SV"))
        SL = ctx.enter_context(nc.semaphore("SL"))
        SO = ctx.enter_context(nc.semaphore("SO"))
        SO2 = ctx.enter_context(nc.semaphore("SO2"))
        block = ctx.enter_context(nc.Block())

        def lt_slot_ap(t):
            return PL2[:, 0:2] if t % 2 == 0 else PL2[:, 2:4]

        class W:
            def __init__(self, eng):
                self.eng = eng
                self.seen = {}

            def need(self, sem, val):
                if val > self.seen.get(id(sem), -1):
                    self.eng.wait_ge(sem, val)
                    self.seen[id(sem)] = val

        @block.sync
        def _(sync):
            w = W(sync)
            sync.dma_start(out=xq, in_=xq_d[:].bitcast(f32r)).then_inc(SDq, 16)
            sync.dma_start(
                out=xc[:, 0:1024], in_=xc_d[:, 0:1024].bitcast(f32r)
            ).then_inc(SDb, 16)
            sync.dma_start(out=minh, in_=minh_d[:]).then_inc(SDm, 16)
            sync.dma_start(
                out=xc[:, 1024:2048], in_=xc_d[:, 1024:2048].bitcast(f32r)
            ).then_inc(SDb2, 16)
            for dram, sb in ((mfg_d, mfg), (w12T_d, w12T)):
                sync.dma_start(out=sb, in_=dram[:].bitcast(f32r)).then_inc(SDa, 16)
            sync.dma_start(out=w2T, in_=w2T_d[:]).then_inc(SDa, 16)
            for dram, sb in ((negv_d, negv), (bslab_d, bslab)):
                sync.dma_start(out=sb, in_=dram[:]).then_inc(SDa, 16)
            sync.dma_start(out=ones_f, in_=onesf_d[:].bitcast(f32r)).then_inc(SDa, 16)
            sync.dma_start(out=onesr, in_=onesr_d[:].bitcast(f32r)).then_inc(SDa, 16)
            sync.dma_start(
                out=xc[:, 2048:4096], in_=xc_d[:, 2048:4096].bitcast(f32r)
            ).then_inc(SDc, 16)
            sync.dma_start(
                out=vpt, in_=vptb_d[:].rearrange("(t p) c -> p t c", p=128)
            ).then_inc(SDd, 16)
            sync.dma_start(
                out=xtb, in_=xtb_d[:].rearrange("(t p) c -> p t c", p=128)
            ).then_inc(SDe, 16)
            for r in range(repeat):
                vv = 2 + r * V_TOT
                ll = r * L_TOT
                for jc in range(8):
                    w.need(SV, vv + v_val[("obs", jc)])
                    sync.dma_start(
                        out=out_chan_d[:, jc * 512 : (jc + 1) * 512],
                        in_=obs[:, jc],
                    ).then_inc(SO, 16)
                w.need(SV, vv + v_val[("sadd", 0)])
                sync.dma_start(
                    out=out_slab_d[:, 0:512], in_=slab[:, 0:512]
                ).then_inc(SO2, 16)
                w.need(SL, ll + l_val[("sadd1", 0)])
                sync.dma_start(
                    out=out_slab_d[:, 512:1024], in_=slab[:, 512:1024]
                ).then_inc(SO2, 16)


        @block.tensor
        def _(pe):
            w = W(pe)
            # p-state warmup: f32 junk matmuls (4 cyc/row) burn the ramp so
            # cqk0 runs at full clock
            w.need(SV, 1)
            for u in range(2):
                m = pe.matmul(
                    PL1[:, u, :],
                    wbuf[:, 0:128],
                    wbuf[:, 128:640],
                    start=True, stop=True,
                )
            m.then_inc(SP_, 1)
            for r in range(repeat):
                aa = 1 + r * A_TOT
                vv = 2 + r * V_TOT
                ll = r * L_TOT
                if r > 0:
                    w.need(SV, vv)
                    w.need(SL, ll)
                    w.need(SA, aa)
                for key in pe_seq:
                    kind, idx = key
                    if kind == "cqk":
                        s = idx
                        half, it = s // 8, s % 8
                        if s == 0:
                            w.need(SDq, 16)
                            w.need(SDb, 16)
                        elif s == 1:
                            w.need(SP_, 1)  # warmup junk out of PL1
                        elif s == 8:
                            w.need(SDc, 16)
                        if s >= 2:
                            w.need(SA, aa + a_val[("cexp", s - 2)])
                        bank = PL2 if s % 2 == 0 else PL1
                        for u in range(4):
                            if s == 0 and u == 2:
                                w.need(SDb2, 16)
                            j0 = half * 2048 + u * 512
                            m = pe.matmul(
                                bank[:, u, :],
                                xq[:, it * 128 : (it + 1) * 128],
                                xc[:, j0 : j0 + 512],
                                start=True, stop=True,
                            )
                        m.then_inc(SP_, 1)
                    elif kind == "qprime":
                        w.need(SDa, 112)
                        w.need(SA, aa + a_val[("cexp", NST - 2)])
                        for h in range(2):
                            m = pe.matmul(
                                PL2[:, h, :], mfg, xq[:, h * 512 : (h + 1) * 512],
                                start=True, stop=True,
                            )
                        m.then_inc(SP_, 1)
                    elif kind == "w12":
                        w.need(SA, aa + a_val[("cexp", NST - 1)])
                        for h in range(2):
                            m = pe.matmul(
                                PL1[:, 2 + h, :], w12T, xq[:, h * 512 : (h + 1) * 512],
                                start=True, stop=True,
                            )
                        m.then_inc(SP_, 1)
                    elif kind == "lt":
                        t = idx
                        if t == 0:
                            w.need(SV, vv + v_val[("qsb", 0)])
                        if t >= 2:
                            w.need(SA, aa + a_val[("pexp", t - 2)])
                        slot = lt_slot_ap(t)
                        for h in range(2):
                            m = pe.matmul(
                                slot[:, h, :],
                                xc[:, t * 128 : (t + 1) * 128],
                                qsb[:, h * 512 : (h + 1) * 512],
                                start=True, stop=True,
                            )
                        m.then_inc(SP_, 1)
                    elif kind == "cava":
                        jc = idx
                        if jc == 0:
                            w.need(SV, vv + v_val[("xnt", NIT - 1)])
                        if jc >= 2:
                            w.need(SV, vv + v_val[("cacc", jc - 2)])
                        for it in range(4):
                            m = pe.matmul(
                                PL1[:, jc % 2, :],
                                xnt[:, it],
                                chanP[:, it, jc * 512 : (jc + 1) * 512],
                                start=(it == 0), stop=False,
                            )
                        m.then_inc(SP_, 1)
                    elif kind == "cavb":
                        jc = idx
                        for it in range(4, NIT):
                            m = pe.matmul(
                                PL1[:, jc % 2, :],
                                xnt[:, it],
                                chanP[:, it, jc * 512 : (jc + 1) * 512],
                                start=False, stop=(it == NIT - 1),
                            )
                        m.then_inc(SP_, 1)
                    elif kind == "w2":
                        jc = idx
                        w.need(SV, vv + v_val[("cacc", jc)])
                        if jc == 0:
                            w.need(SV, vv + v_val[("slab0", 0)])
                        if jc >= 2:
                            w.need(SV, vv + v_val[("obs", jc - 2)])
                        m = pe.matmul(
                            PL1[:, 2 + jc % 2, :], w2T,
                            chanacc[:, jc * 512 : (jc + 1) * 512],
                            start=True, stop=True,
                        )
                        m.then_inc(SP_, 1)
                    elif kind == "pav":
                        t = idx
                        w.need(SA, aa + a_val[("pexp", t)])
                        if t == 0:
                            w.need(SV, vv + v_val[("cacc", 7)])
                            w.need(SDd, 16)
                        for h in range(2):
                            m = pe.matmul(
                                PL1[:, h, :],
                                vpt[:, t],
                                ptb[:, t, h * 512 : (h + 1) * 512],
                                start=(t == 0), stop=(t == NJT - 1),
                            )
                        m.then_inc(SP_, 1)
                    elif kind == "rred":
                        w.need(SL, ll + l_val[("racc", NJT - 1)])
                        w.need(SV, vv + v_val[("obs", 7)])
                        for h in range(2):
                            m = pe.matmul(
                                PL1[0:1, 2 + h, :], ones_f,
                                racc_p[:, h * 512 : (h + 1) * 512],
                                start=True, stop=True,
                            )
                        m.then_inc(SP_, 1)
                    elif kind == "rrep":
                        w.need(SV, vv + v_val[("recip", 0)])
                        w.need(SA, aa + a_val[("pexp", 30)])
                        for h in range(2):
                            m = pe.matmul(
                                PL2[:, h, :], onesr,
                                rrec_f[0:1, h * 512 : (h + 1) * 512],
                                start=True, stop=True,
                            )
                        m.then_inc(SP_, 1)

        @block.scalar
        def _(act):
            w = W(act)
            w.need(SV, 2)
            act.activation(
                wbufa[:, 0:1], wbufa[:, 1:2],
                Exp, bias=wbufa[:, 2:3],
            ).then_inc(SA, 1)
            for r in range(repeat):
                pp = 1 + r * P_TOT
                aa = 1 + r * A_TOT
                vv = 2 + r * V_TOT
                ll = r * L_TOT
                if r > 0:
                    w.need(SV, vv)
                    w.need(SL, ll)
                    w.need(SP_, pp)
                for key in act_seq:
                    kind, idx = key
                    if kind == "cexp":
                        s = idx
                        half, it = s // 8, s % 8
                        if s == 0:
                            w.need(SDm, 16)
                        w.need(SP_, pp + p_val[("cqk", s)])
                        bank = PL2 if s % 2 == 0 else PL1
                        act.activation(
                            chanP[:, it, half * 2048 : (half + 1) * 2048],
                            flat(bank),
                            Exp,
                            bias=minh[:, it : it + 1],
                            accum_out=rs2[:, it, half : half + 1],
                        ).then_inc(SA, 1)
                    elif kind == "pexp":
                        t = idx
                        if t == 0:
                            w.need(SDa, 112)
                        w.need(SP_, pp + p_val[("lt", t)])
                        slot = lt_slot_ap(t)
                        act.activation(
                            ptb[:, t], flat(slot), Exp,
                            bias=negv[:, t : t + 1],
                        ).then_inc(SA, 1)
                    else:  # rrsb1
                        w.need(SP_, pp + p_val[("rrep", 0)])
                        act.activation(
                            rrsb[:, 512:1024], PL2[:, 1, :],
                            mybir.ActivationFunctionType.Copy,
                        ).then_inc(SA, 1)

        @block.vector
        def _(dve):
            w = W(dve)
            dve.memset(wbuf, 0.0).then_inc(SV, 1)
            dve.memset(wbufa, 0.0).then_inc(SV, 1)
            for r in range(repeat):
                pp = 1 + r * P_TOT
                aa = 1 + r * A_TOT
                vv = 2 + r * V_TOT
                ll = r * L_TOT
                if r > 0:
                    w.need(SO, r * O_PER)
                    w.need(SO2, r * 32)
                    w.need(SP_, pp)
                    w.need(SL, ll)
                for key in dve_seq:
                    kind, idx = key
                    if kind == "qsb":
                        w.need(SP_, pp + p_val[("qprime", 0)])
                        dve.tensor_copy(qsb, flat(PL2[:, 0:2])).then_inc(SV, 1)
                    elif kind == "rs":
                        w.need(SA, aa + a_val[("cexp", NST - 1)])
                        dve.tensor_reduce(
                            out=rc, in_=rs2, axis=X, op=add
                        ).then_inc(SV, 1)
                    elif kind == "rcr":
                        w.need(SV, vv + v_val[("rs", 0)])
                        dve.reciprocal(out=rcr, in_=rc).then_inc(SV, 1)
                    elif kind == "xnt":
                        t = idx
                        if t == 0:
                            w.need(SDe, 16)
                            w.need(SV, vv + v_val[("rcr", 0)])
                        dve.tensor_scalar_mul(
                            xnt[:, t], xtb[:, t], rcr[:, t : t + 1]
                        ).then_inc(SV, 1)
                    elif kind == "slab0":
                        w.need(SP_, pp + p_val[("w12", 0)])
                        w.need(SDa, 112)
                        dve.tensor_scalar_add(
                            slab0, flat(PL1[:, 2:4]), bslab
                        ).then_inc(SV, 1)
                    elif kind == "cacc":
                        jc = idx
                        w.need(SP_, pp + p_val[("cavb", jc)])
                        dve.tensor_copy(
                            chanacc[:, jc * 512 : (jc + 1) * 512], PL1[:, jc % 2, :]
                        ).then_inc(SV, 1)
                    elif kind == "obs":
                        jc = idx
                        w.need(SP_, pp + p_val[("w2", jc)])
                        dve.tensor_copy(
                            obs[:, jc], PL1[:, 2 + jc % 2, :]
                        ).then_inc(SV, 1)
                    elif kind == "recip":
                        w.need(SP_, pp + p_val[("rred", 0)])
                        with nc.allow_low_precision(reason="f32r recip for PE"):
                            dve.reciprocal(
                                out=rrec_f, in_=flat(PL1[0:1, 2:4])
                            ).then_inc(SV, 1)
                    elif kind == "rrsb":
                        w.need(SP_, pp + p_val[("rrep", 0)])
                        dve.tensor_copy(rrsb[:, 0:512], PL2[:, 0, :]).then_inc(SV, 1)
                    elif kind == "smul":
                        w.need(SV, vv + v_val[("rrsb", 0)])
                        dve.tensor_tensor(
                            out=slab[:, 0:512], in0=PL1[:, 0, :],
                            in1=rrsb[:, 0:512], op=mult,
                        ).then_inc(SV, 1)
                    elif kind == "smul1":
                        w.need(SA, aa + a_val[("rrsb1", 0)])
                        dve.tensor_tensor(
                            out=slab[:, 512:1024], in0=PL1[:, 1, :],
                            in1=rrsb[:, 512:1024], op=mult,
                        ).then_inc(SV, 1)
                    elif kind == "sadd":
                        w.need(SV, vv + v_val[("smul", 0)])
                        w.need(SV, vv + v_val[("slab0", 0)])
                        dve.tensor_tensor(
                            out=slab[:, 0:512], in0=slab[:, 0:512],
                            in1=slab0[:, 0:512], op=add,
                        ).then_inc(SV, 1)

        @block.gpsimd
        def _(pool):
            w = W(pool)
            for r in range(repeat):
                pp = 1 + r * P_TOT
                aa = 1 + r * A_TOT
                vv = 2 + r * V_TOT
                ll = r * L_TOT
                oo = r * O_PER
                if r > 0:
                    w.need(SO, oo)
                    w.need(SO2, r * 32)
                    w.need(SP_, pp)
                    w.need(SV, vv)
                for key in pool_seq:
                    kind, idx = key
                    if kind == "racc":
                        t = idx
                        w.need(SA, aa + a_val[("pexp", t)])
                        if t == 0:
                            pool.tensor_copy(racc_p, ptb[:, 0]).then_inc(SL, 1)
                        else:
                            w.need(SL, ll + l_val[("racc", t - 1)])
                            pool.tensor_tensor(
                                out=racc_p, in0=racc_p, in1=ptb[:, t], op=add
                            ).then_inc(SL, 1)
                    elif kind == "sadd1":
                        w.need(SV, vv + v_val[("smul1", 0)])
                        pool.tensor_tensor(
                            out=slab[:, 512:1024], in0=slab[:, 512:1024],
                            in1=slab0[:, 512:1024], op=add,
                        ).then_inc(SL, 1)

    return nc


def _prep_inputs(x, f_w, f_b, g_w, g_b, h_w, h_b, out_w, out_b):
    f32 = np.float32
    x = np.ascontiguousarray(np.asarray(x, dtype=f32))
    B = x.shape[0]
    x2 = x.reshape(B, C, HW)
    f_w = np.asarray(f_w, f32)
    f_b = np.asarray(f_b, f32)
    g_w = np.asarray(g_w, f32)
    g_b = np.asarray(g_b, f32)
    h_w = np.asarray(h_w, f32)
    h_b = np.asarray(h_b, f32)
    out_w = np.asarray(out_w, f32)
    out_b = np.asarray(out_b, f32)
    W1 = out_w[:, :C]
    W2 = out_w[:, C:]
    hW1 = W1 @ h_w
    shared = {
        "mfg": np.ascontiguousarray(f_w.T @ g_w),
        "w12T": np.ascontiguousarray((W1 + W2).T),
        "w2T": np.ascontiguousarray(W2.T.astype(bfloat16)),
        "bslab": (W1 @ h_b + out_b).reshape(C, 1).copy(),
        "ones_f": np.ones((128, 1), f32),
        "ones_r": np.ones((1, 128), f32),
    }
    in_maps = []
    for core in range(8):
        b, q = core // 4, core % 4
        xcv = np.ascontiguousarray(x2[b])
        xqv = np.ascontiguousarray(xcv[:, q * NQ : (q + 1) * NQ])
        v = f_b @ (g_w @ xcv) + float(f_b @ g_b)
        negv = np.ascontiguousarray((v - POS_OFF).reshape(NJT, 128).T.astype(f32))
        d = np.einsum("ci,ci->i", xqv, xqv)
        minh = np.ascontiguousarray((-d).reshape(NIT, 128).T.astype(f32))
        in_maps.append({
            "xq": xqv,
            "xc": xcv,
            "vptb": np.ascontiguousarray((hW1 @ xcv).T.astype(bfloat16)),
            "xtb": np.ascontiguousarray(xqv.T.astype(bfloat16)),
            "negv": negv,
            "minh": minh,
            **shared,
        })
    return in_maps


def _combine(results, B):
    y = np.zeros((B, C, HW), np.float32)
    for core in range(8):
        b, q = core // 4, core % 4
        y[b, :, q * NQ : (q + 1) * NQ] += results[core]["out_slab"]
        y[b] += results[core]["out_chan"]
    return y.reshape(B, C, 64, 64)


def run_on_hw(in_maps, trace=False):
    from concourse.bass_utils import run_bass_kernel_spmd

    if "nc" not in _CACHE:
        _CACHE["nc"] = _build_bass()
    return run_bass_kernel_spmd(_CACHE["nc"], in_maps, list(range(8)), trace=trace)


def kernel(x, f_w, f_b, g_w, g_b, h_w, h_b, out_w, out_b):
    in_maps = _prep_inputs(x, f_w, f_b, g_w, g_b, h_w, h_b, out_w, out_b)
    res = run_on_hw(in_maps)
    return _combine(res.results, np.asarray(x).shape[0])

